# revision 3
# baseline (speedup 1.0000x reference)
"""Mixtral-style MoE layer (E=8 experts, top-2, capacity-dropped) on 8 Trainium2
NeuronCores.

Strategy (expert-parallel, per sharding hint):
  - Router + top-k + capacity dispatch run on host, bit-identical to the
    reference (same jax-CPU ops), producing per-expert [capacity] token/gate
    dispatch buffers.
  - Core e runs expert e's FFN on its [1280, 1024] dispatched tokens:
    h = gelu(inpT.T @ w1 + b1); y = (h @ w2 + b2) * gate.
    Data is fp32; matmuls run as float32r (full-rate PE, ~1e-4 rounding).
    The compiler requires fp32r matmul operands to be produced by a rounding
    compute op, so DMA'd fp32 tiles pass through GpSimd/ACT copy converts.
  - Host scatter-adds the [E, cap, C] expert outputs back to token order and
    computes the aux load-balancing loss (exact, from router probs).

Device kernel layout per core:
  - inpT [C, cap] resident in SBUF as f32r (contract dim on partitions).
  - H processed in 8 blocks of 512: w1/w2 streamed once, h.T block kept in
    SBUF (f32r, produced by the gelu ACT op), second matmul accumulated into
    an SBUF fp32 y accumulator via DVE adds.
"""

import os

os.environ.setdefault("JAX_PLATFORMS", "")  # don't let a cpu-only pin hide axon

import numpy as np

# ---- problem constants (hardcoded per contract) ----
E, TOPK, MULT, CF = 8, 2, 4, 1.25
B, T, C = 4, 2048, 1024
N = B * T                      # 8192 tokens
H = C * MULT                   # 4096
CAP = int(CF * N / E)          # 1280 slots per expert
P = 128
CC = C // P                    # 8 contraction chunks
G = H // P                     # 32 h chunks
HB = 8                         # h blocks
HBG = G // HB                  # 4 h chunks per block
TT = CAP // P                  # 10 token tiles
SEGS = ((0, 512), (512, 512), (1024, 256))  # mm1 moving-dim segments

_CACHE = {}


def _build_nc():
    """Build + compile the per-core Bass program (identical on all 8 cores)."""
    from contextlib import ExitStack

    import concourse.bacc as bacc
    import concourse.tile as tile
    from concourse import mybir

    f32 = mybir.dt.float32
    f32r = mybir.dt.float32r
    Gelu = mybir.ActivationFunctionType.Gelu

    nc = bacc.Bacc("TRN2", target_bir_lowering=False, debug=False, num_devices=8)

    inpT_d = nc.dram_tensor("inpT", [C, CAP], f32, kind="ExternalInput")
    w1_d = nc.dram_tensor("w1", [C, H], f32, kind="ExternalInput")
    w2_d = nc.dram_tensor("w2", [H, C], f32, kind="ExternalInput")
    b1_d = nc.dram_tensor("b1", [H], f32, kind="ExternalInput")
    b2_d = nc.dram_tensor("b2", [C], f32, kind="ExternalInput")
    gate_d = nc.dram_tensor("gate", [CAP], f32, kind="ExternalInput")
    y_d = nc.dram_tensor("y", [CAP, C], f32, kind="ExternalOutput")

    inpT_ap = inpT_d.ap().rearrange("(cc p) t -> p cc t", p=P)
    w1_ap = w1_d.ap().rearrange("(cc p) h -> p cc h", p=P)
    w2_ap = w2_d.ap().rearrange("(g p) c -> p g c", p=P)

    with tile.TileContext(nc) as tc, ExitStack() as ctx:
        const = ctx.enter_context(tc.tile_pool(name="const", bufs=1))
        stage = ctx.enter_context(tc.tile_pool(name="stage", bufs=3))
        w1sp = ctx.enter_context(tc.tile_pool(name="w1sp", bufs=2))
        w1p = ctx.enter_context(tc.tile_pool(name="w1p", bufs=3))
        w2sp = ctx.enter_context(tc.tile_pool(name="w2sp", bufs=2))
        w2p = ctx.enter_context(tc.tile_pool(name="w2p", bufs=2))
        htp = ctx.enter_context(tc.tile_pool(name="htp", bufs=2))
        outp = ctx.enter_context(tc.tile_pool(name="outp", bufs=2))
        ps1 = ctx.enter_context(tc.tile_pool(name="ps1", bufs=2, space="PSUM"))
        ps2 = ctx.enter_context(tc.tile_pool(name="ps2", bufs=2, space="PSUM"))

        # ---- resident inputs: DMA f32 staging -> rounded f32r tiles ----
        inpT_r = const.tile([P, CC, CAP], f32r)
        for cc in range(CC):
            st = stage.tile([P, CAP], f32, tag="st")
            nc.sync.dma_start(out=st, in_=inpT_ap[:, cc, :])
            nc.gpsimd.tensor_copy(inpT_r[:, cc, :], st)

        b1_sb = const.tile([P, G], f32)
        nc.sync.dma_start(out=b1_sb, in_=b1_d.ap().rearrange("(g p) -> p g", p=P))
        gate_sb = const.tile([P, TT], f32)
        nc.sync.dma_start(out=gate_sb, in_=gate_d.ap().rearrange("(tt p) -> p tt", p=P))
        b2bc = const.tile([P, C], f32)
        nc.sync.dma_start(out=b2bc, in_=b2_d.ap().unsqueeze(0).broadcast_to([P, C]))
        y_acc = const.tile([P, TT, C], f32)

        for blk in range(HB):
            # w2 block: stage per h-chunk, ACT-convert to f32r
            w2r = w2p.tile([P, HBG, C], f32r)
            for hh in range(HBG):
                g = blk * HBG + hh
                w2st = w2sp.tile([P, C], f32, tag="w2st")
                nc.sync.dma_start(out=w2st, in_=w2_ap[:, g, :])
                nc.scalar.activation(
                    out=w2r[:, hh, :], in_=w2st,
                    func=mybir.ActivationFunctionType.Copy, bias=0.0, scale=1.0,
                )

            hT = htp.tile([P, HBG, CAP], f32r)
            for hh in range(HBG):
                g = blk * HBG + hh
                w1st = w1sp.tile([P, CC, P], f32, tag="w1st")
                nc.sync.dma_start(out=w1st, in_=w1_ap[:, :, g * P : (g + 1) * P])
                w1r = w1p.tile([P, CC, P], f32r)
                nc.gpsimd.tensor_copy(w1r, w1st)

                pa = ps1.tile([P, 512], f32, tag="pa")
                pb = ps1.tile([P, 512], f32, tag="pb")
                pc = ps1.tile([P, 256], f32, tag="pc")
                psums = (pa, pb, pc)
                for cc in range(CC):
                    lhsT = w1r[:, cc, :]
                    for (off, ln), pt in zip(SEGS, psums):
                        nc.tensor.matmul(
                            pt,
                            lhsT,
                            inpT_r[:, cc, off : off + ln],
                            start=(cc == 0),
                            stop=(cc == CC - 1),
                        )
                for (off, ln), pt in zip(SEGS, psums):
                    nc.scalar.activation(
                        out=hT[:, hh, off : off + ln],
                        in_=pt,
                        func=Gelu,
                        bias=b1_sb[:, g : g + 1],
                        scale=1.0,
                    )

            for tt in range(TT):
                for ch in range(2):
                    p2 = ps2.tile([P, 512], f32)
                    for hh in range(HBG):
                        nc.tensor.matmul(
                            p2,
                            hT[:, hh, tt * P : (tt + 1) * P],
                            w2r[:, hh, ch * 512 : (ch + 1) * 512],
                            start=(hh == 0),
                            stop=(hh == HBG - 1),
                        )
                    dst = y_acc[:, tt, ch * 512 : (ch + 1) * 512]
                    if blk == 0:
                        nc.vector.tensor_copy(dst, p2)
                    else:
                        nc.vector.tensor_add(dst, dst, p2)

        for tt in range(TT):
            o = outp.tile([P, C], f32, tag="o")
            nc.vector.tensor_add(o, y_acc[:, tt, :], b2bc)
            nc.vector.tensor_scalar_mul(o, o, gate_sb[:, tt : tt + 1])
            nc.sync.dma_start(out=y_d.ap()[tt * P : (tt + 1) * P, :], in_=o)

    nc.compile()
    return nc


def _route_host(x, rw, rb):
    """Routing bit-identical to the reference (same jax ops, CPU)."""
    import jax
    import jax.numpy as jnp

    cpu = jax.devices("cpu")[0]
    with jax.default_device(cpu):
        xf = jnp.asarray(np.asarray(x)).reshape(N, C)
        logits = xf @ jnp.asarray(np.asarray(rw)).T + jnp.asarray(np.asarray(rb))
        probs = jax.nn.softmax(logits, axis=-1)
        topk_probs, topk_idx = jax.lax.top_k(probs, TOPK)
        topk_probs = topk_probs / topk_probs.sum(-1, keepdims=True)
        flat_idx = np.asarray(topk_idx).reshape(-1).astype(np.int64)
        flat_gate = np.asarray(topk_probs).reshape(-1).astype(np.float32)
        probs_np = np.asarray(probs)
    return flat_idx, flat_gate, probs_np


def _dispatch(flat_idx, flat_gate):
    """First-come-first-served capacity assignment, exact integer math."""
    NK = N * TOPK
    order = np.argsort(flat_idx, kind="stable")
    sorted_e = flat_idx[order]
    grp_start = np.searchsorted(sorted_e, np.arange(E))
    pos = np.empty(NK, dtype=np.int64)
    pos[order] = np.arange(NK) - grp_start[sorted_e]
    counts = np.bincount(flat_idx, minlength=E)

    keep = pos < CAP
    tok = np.arange(NK) // TOPK
    buf = flat_idx * CAP + pos  # valid where keep

    tok_buf = np.zeros(E * CAP, dtype=np.int64)
    gate_buf = np.zeros(E * CAP, dtype=np.float32)
    tok_buf[buf[keep]] = tok[keep]
    gate_buf[buf[keep]] = flat_gate[keep]
    return keep, buf, tok, tok_buf, gate_buf, counts


def kernel(x, rw, rb, w1, b1, w2, b2):
    from concourse.bass_utils import run_bass_kernel_spmd

    x = np.ascontiguousarray(np.asarray(x, dtype=np.float32))
    rw = np.asarray(rw, dtype=np.float32)
    rb = np.asarray(rb, dtype=np.float32)
    w1 = np.ascontiguousarray(np.asarray(w1, dtype=np.float32))
    b1 = np.ascontiguousarray(np.asarray(b1, dtype=np.float32))
    w2 = np.ascontiguousarray(np.asarray(w2, dtype=np.float32))
    b2 = np.ascontiguousarray(np.asarray(b2, dtype=np.float32))

    flat_idx, flat_gate, probs = _route_host(x, rw, rb)
    keep, buf, tok, tok_buf, gate_buf, counts = _dispatch(flat_idx, flat_gate)

    xf = x.reshape(N, C)
    in_maps = []
    for e in range(E):
        xg = xf[tok_buf[e * CAP : (e + 1) * CAP]]           # [CAP, C]
        in_maps.append(
            {
                "inpT": np.ascontiguousarray(xg.T),          # [C, CAP]
                "w1": w1[e],
                "w2": w2[e],
                "b1": b1[e],
                "b2": b2[e],
                "gate": gate_buf[e * CAP : (e + 1) * CAP],
            }
        )

    if "nc" not in _CACHE:
        _CACHE["nc"] = _build_nc()
    res = run_bass_kernel_spmd(_CACHE["nc"], in_maps, list(range(E)))
    _CACHE["in_maps"] = in_maps

    out_buf = np.empty((E * CAP, C), dtype=np.float32)
    for e in range(E):
        out_buf[e * CAP : (e + 1) * CAP] = res.results[e]["y"]

    y = np.zeros((N, C), dtype=np.float32)
    for k in range(TOPK):
        kk = keep[k::TOPK]
        tk = tok[k::TOPK][kk]
        y[tk] += out_buf[buf[k::TOPK][kk]]

    imp = probs.sum(0)
    imp = imp / imp.sum()
    load = np.minimum(counts.astype(np.float32), CAP)
    load = load / load.sum()
    aux_loss = np.float32((imp * load).sum() * (E**2))

    return y.reshape(B, T, C), aux_loss


def _traced_run():
    """Re-run the last in_maps with NTFF profiling (for test.py)."""
    from concourse.bass_utils import run_bass_kernel_spmd

    return run_bass_kernel_spmd(
        _CACHE["nc"], _CACHE["in_maps"], list(range(E)), trace=True
    )
